# revision 69
# baseline (speedup 1.0000x reference)
"""Block-sparse attention Trainium2 kernel (8 NeuronCores, SPMD).

Sharding: data-parallel over (batch, head-group): core c handles batch b=c//4
and heads [4*(c%4) .. 4*(c%4)+4). Block index lists are replicated (used
host-side to build the static program). Each core returns a partial
[S, E] output (its heads' contribution through Wo); the host sums the 4
partials per batch (the unshard step of the head-sharded GEMM).

Pipeline per core (all on device):
  x -> PE-transpose -> x^T -> QKV projection (weights stationary) giving
  Q^T,K^T [d,s] (f32) and V^T -> PE-transpose -> V [s,d] (bf16).
  Per head-pair (2 heads packed on 128 partitions):
    Phase A (per row-block i): scores = Q_i^T.T @ K^T runs (PSUM) -> exp (ACT)
      -> per-block denom (DVE 3D reduce) -> recip -> normalize (GPSIMD, bf16)
      -> PE-transpose 64x64 blocks -> attnT storage (bf16).
    Phase B: out^T accumulated in PSUM via V-stationary matmuls over attnT.
  Wo projection from out^T tiles (stationary) + rank-1 bias add; partial out
  DMA'd back.
"""
import numpy as np

B, S, E, H, D, BS = 2, 2048, 1024, 16, 64, 64
NB = S // BS          # 32
NCORES = 8
HPC = 4               # heads per core
NPAIRS = 628          # length of block index lists
MAXW = 16             # attnT slots reserved per row block (fixed stride)

LAST_RESULTS = None   # BassKernelResults of the most recent run (for test.py)


# ---------------------------------------------------------------- host planning

def _plan(block_rows, block_cols):
    """Static schedule shared by every head/core.

    Row layout (compact positions, for scores/exp): pairs first (each 2
    blocks), then singles ascending.  attnT window layout: pairs at the
    same positions; single k expands to 2 blocks (real + dead) so each
    entry is one aligned 128-col xbar window -> ONE dma transpose per row.

    Returns per-row structures:
      J[i]            sorted col-block list
      entries[i]      ('pair', j_even) then ('single', j) ascending
      score_chunks[i] <=8 compact blocks each: jlist, npr (pairs in chunk),
                      e0 (first entry idx), p0 (first compact position)
      col[i][e]       attnT slot (128-col window) of entry e
      nP[i], nS[i]    pair / single counts
      ncols_total     total slots
    """
    J = [[] for _ in range(NB)]
    for r, c in zip(np.asarray(block_rows).tolist(), np.asarray(block_cols).tolist()):
        J[int(r)].append(int(c))
    for i in range(NB):
        J[i].sort()

    entries = [[] for _ in range(NB)]
    score_chunks = [[] for _ in range(NB)]
    nP = [0] * NB
    nSE = [0] * NB
    nSO = [0] * NB
    for i in range(NB):
        js = J[i]
        pairs, sE, sO = [], [], []
        t = 0
        while t < len(js):
            if t + 1 < len(js) and js[t] % 2 == 0 and js[t + 1] == js[t] + 1:
                pairs.append(('pair', js[t]))
                t += 2
            elif js[t] % 2 == 0:
                sE.append(('se', js[t]))
                t += 1
            else:
                sO.append(('so', js[t]))
                t += 1
        nP[i], nSE[i], nSO[i] = len(pairs), len(sE), len(sO)
        walk = pairs + sE + sO
        entries[i] = walk
        # score chunks over the compact layout, <=8 blocks, pair-aligned
        cur, cur_blocks = [], 0
        sc_list = []
        for ent in walk:
            nb = 2 if ent[0] == 'pair' else 1
            if cur_blocks + nb > 8:
                sc_list.append(cur)
                cur, cur_blocks = [], 0
            cur.append(ent)
            cur_blocks += nb
        if cur:
            sc_list.append(cur)
        e0 = 0
        p0 = 0
        for sc in sc_list:
            jlist = []
            npr = sum(1 for e in sc if e[0] == 'pair')
            nse = sum(1 for e in sc if e[0] == 'se')
            nso = sum(1 for e in sc if e[0] == 'so')
            for ent in sc:
                jlist.append(ent[1])
                if ent[0] == 'pair':
                    jlist.append(ent[1] + 1)
            score_chunks[i].append(dict(jlist=jlist, npr=npr, nse=nse, nso=nso,
                                        e0=e0, p0=p0))
            e0 += len(sc)
            p0 += len(jlist)
        assert len(walk) <= MAXW, (i, len(walk))

    # fixed row stride MAXW slots: slot(i, e) = i*MAXW + e
    col = [dict() for _ in range(NB)]
    for i in range(NB):
        for e_idx, ent in enumerate(entries[i]):
            col[i][e_idx] = i * MAXW + e_idx
    return dict(J=J, entries=entries, score_chunks=score_chunks, col=col,
                nP=nP, nSE=nSE, nSO=nSO, ncols_total=NB * MAXW)


def _runs(jlist):
    """Maximal consecutive runs [(j0, n), ...] in a sorted j list."""
    runs = []
    for j in jlist:
        if runs and j == runs[-1][0] + runs[-1][1]:
            runs[-1][1] += 1
        else:
            runs.append([j, 1])
    return [(a, b) for a, b in runs]


# ---------------------------------------------------------------- bass program

def _build_program(plan, stage='full'):
    import concourse.bacc as bacc
    import concourse.mybir as mybir
    from concourse.tile import TileContext
    from concourse import masks

    F32 = mybir.dt.float32
    F32R = mybir.dt.float32r
    BF16 = mybir.dt.bfloat16
    AF = mybir.ActivationFunctionType
    ALU = mybir.AluOpType
    AX = mybir.AxisListType

    nc = bacc.Bacc("TRN2", target_bir_lowering=False, debug=False)

    x_in = nc.dram_tensor("x_local", [S, E], F32, kind="ExternalInput")
    wqkv_in = nc.dram_tensor("w_qkv", [E, 3 * HPC * D], BF16, kind="ExternalInput")
    bqkv_in = nc.dram_tensor("b_qkv", [3 * HPC * D], F32, kind="ExternalInput")
    wo_in = nc.dram_tensor("w_o", [HPC * D, E], BF16, kind="ExternalInput")
    y_out = nc.dram_tensor("y_partial", [S, E], F32, kind="ExternalOutput")

    NT = 3 * HPC * D // 128      # 6 qkv n-tiles
    KT = E // 128                # 8 contraction tiles
    ST = S // 128                # 16 s tiles

    ncols_total = plan['ncols_total']
    entries, score_chunks, col = plan['entries'], plan['score_chunks'], plan['col']
    J = plan['J']

    with TileContext(nc) as tc:
        with tc.tile_pool(name="const", bufs=1) as cpool, \
             tc.tile_pool(name="qkvT", bufs=1) as qpool, \
             tc.tile_pool(name="vnorm", bufs=1) as vpool, \
             tc.tile_pool(name="outsb", bufs=1) as opool:

            idf = cpool.tile([128, 128], F32)
            masks.make_identity(nc, idf[:])
            idb = cpool.tile([128, 128], BF16)
            masks.make_identity(nc, idb[:])
            ones_t = cpool.tile([1, 128], BF16)
            nc.vector.memset(ones_t[:], 1.0)
            zrow = cpool.tile([1, 64], BF16)
            nc.vector.memset(zrow[:], 0.0)
            bqkv_sb = cpool.tile([128, NT], F32)
            nc.sync.dma_start(bqkv_sb[:], bqkv_in.ap().rearrange("(t p) -> p t", p=128))
            bsc = cpool.tile([128, NT], F32)
            # q biases (tiles 0,1) pre-scaled by 1/sqrt(D)
            nc.scalar.mul(bsc[:, 0:2], bqkv_sb[:, 0:2], 1.0 / float(np.sqrt(D)))
            nc.scalar.copy(bsc[:, 2:NT], bqkv_sb[:, 2:NT])


            # ---- x -> x^T ----------------------------------------------------
            qkvT = [qpool.tile([128, S], BF16, name=f"qkvT{t}", tag=f"qkvT{t}") for t in range(NT)]
            trctx = tc.tile_pool(name="tr_ps", bufs=2, space="PSUM")
            ps_tr = trctx.__enter__()
            with tc.tile_pool(name="xload", bufs=3) as xpool, \
                 tc.tile_pool(name="xT", bufs=1) as xtp, \
                 tc.tile_pool(name="wq", bufs=1) as wpool:
                xT = [xtp.tile([128, S], BF16, name=f"xT{k}", tag=f"xT{k}") for k in range(KT)]
                for m in range(ST):
                    xs = xpool.tile([128, E], F32, tag="xs")
                    nc.sync.dma_start(xs[:], x_in.ap()[m * 128:(m + 1) * 128, :])
                    xb = xpool.tile([128, E], BF16, tag="xb")
                    nc.vector.tensor_copy(xb[:], xs[:])
                    for k in range(KT):
                        tp = ps_tr.tile([128, 512], BF16, tag="tr")
                        nc.tensor.transpose(tp[:, 0:128], xb[:, k * 128:(k + 1) * 128],
                                            idb[:])
                        eng = nc.vector if (k % 2 == 0) else nc.scalar
                        if eng is nc.vector:
                            nc.vector.tensor_copy(xT[k][:, m * 128:(m + 1) * 128],
                                                  tp[:, 0:128])
                        else:
                            nc.scalar.copy(xT[k][:, m * 128:(m + 1) * 128],
                                           tp[:, 0:128])

                # ---- QKV projection (weights stationary) --------------------
                wsb = [wpool.tile([128, 3 * HPC * D], BF16, name=f"w{k}", tag=f"w{k}")
                       for k in range(KT)]
                for k in range(KT):
                    nc.sync.dma_start(wsb[k][:], wqkv_in.ap()[k * 128:(k + 1) * 128, :])
                with tc.tile_pool(name="pj_ps", bufs=4, space="PSUM") as ps_pj:
                    for t in range(NT):
                        scale = 1.0 / float(np.sqrt(D)) if t < 2 else 1.0
                        for sc in range(S // 512):
                            pt = ps_pj.tile([128, 512], F32, tag="pj")
                            for k in range(KT):
                                nc.tensor.matmul(pt[:],
                                                 wsb[k][:, t * 128:(t + 1) * 128],
                                                 xT[k][:, sc * 512:(sc + 1) * 512],
                                                 start=(k == 0), stop=(k == KT - 1))
                            nc.scalar.activation(qkvT[t][:, sc * 512:(sc + 1) * 512],
                                                 pt[:], AF.Identity,
                                                 bias=bsc[:, t:t + 1], scale=scale)

            # ---- V^T -> V (normal layout, bf16) ------------------------------
            # Full-width transposes (both heads of a pair at once); pair tile
            # c holds keys of block 2c at partitions 0:64 and block 2c+1 at
            # 64:128 (odd-single matmuls read the high half directly).
            V = [vpool.tile([128, NB // 2 * D], BF16, name=f"V{h}", tag=f"V{h}") for h in range(HPC)]
            for vp in range(2):                 # head pairs (0,1) and (2,3)
                vt = qkvT[4 + vp]
                for c4 in range(0, NB // 2, 4):  # 4 s-chunks per psum tile
                    tp = ps_tr.tile([128, 512], BF16, tag="tr")
                    for u in range(4):
                        c = c4 + u
                        nc.tensor.transpose(tp[:, u * 128:(u + 1) * 128],
                                            vt[:, c * 128:(c + 1) * 128], idb[:])
                    for lh in range(2):
                        src = tp[:, 0:512].rearrange("p (n q) -> p n q", q=128)[
                            :, :, lh * 64:(lh + 1) * 64]
                        dst = V[2 * vp + lh][:, c4 * 64:(c4 + 4) * 64].rearrange(
                            "p (n q) -> p n q", q=64)
                        if lh == 0:
                            nc.scalar.copy(dst, src)
                        else:
                            nc.vector.tensor_copy(dst, src)

            trctx.__exit__(None, None, None)   # free transpose PSUM banks
            mmctx = tc.tile_pool(name="mm_ps", bufs=4, space="PSUM")
            ps_mm = mmctx.__enter__()
            otctx = tc.tile_pool(name="ot_ps", bufs=1, space="PSUM")
            ps_out = otctx.__enter__()

            outSB = [opool.tile([128, S], BF16, name=f"outSB{hp}", tag=f"outSB{hp}") for hp in range(2)]

            if stage == 'proj':
                prb = opool.tile([128, E], F32, name="prb")
                nc.vector.tensor_copy(prb[:], qkvT[0][:, 0:E])
                nc.sync.dma_start(y_out.ap()[0:128, :], prb[:])
                nc.vector.tensor_copy(prb[:], V[3][:, 0:E].bitcast(F32).broadcast_to([128, E]) if False else qkvT[5][:, 0:E])
                nc.sync.dma_start(y_out.ap()[128:256, :], prb[:])

            # ---- attention per head pair ------------------------------------
            # attnT storage: one 128-col slot per entry ([2 key-blocks] x
            # [head-a q | head-b q]); slots filled by SBUF->SBUF DMA
            # transposes (xbar), no PE transposes / PSUM copies.
            with tc.tile_pool(name="attnT", bufs=1) as apool:
             if stage != 'proj':
              for hp in range(2):
                ha, hb_ = 2 * hp, 2 * hp + 1
                qT = qkvT[hp]          # [128, S] heads (ha at 0:64, hb at 64:128)
                kT = qkvT[2 + hp]
                aTall = apool.tile([128, ncols_total * 128], BF16,
                                   name=f"aTall{hp}", tag="aTall")
                # block-diagonal Q: qbd[:, i-block] = [[qa_i, 0], [0, qb_i]]
                # so one matmul computes both heads' scores (zeros kill the
                # cross-head contraction terms)
                qbd = apool.tile([128, NB * 128], BF16, name=f"qbd{hp}",
                                 tag="qbd")
                nc.vector.memset(qbd[:], 0.0)
                nc.vector.tensor_copy(
                    qbd[0:64, 0:NB * 128].rearrange(
                        "p (n q) -> p n q", q=128)[:, :, 0:64],
                    qT[0:64, 0:S].rearrange("p (n q) -> p n q", q=64))
                nc.scalar.copy(
                    qbd[64:128, 0:NB * 128].rearrange(
                        "p (n q) -> p n q", q=128)[:, :, 64:128],
                    qT[64:128, 0:S].rearrange("p (n q) -> p n q", q=64))

                with tc.tile_pool(name=f"exp{hp}", bufs=4) as epool, \
                     tc.tile_pool(name=f"att{hp}", bufs=3) as atpool, \
                     tc.tile_pool(name=f"den{hp}", bufs=8) as dpool:
                    # -------- Phase A --------
                    nP, nSE, nSO = plan['nP'], plan['nSE'], plan['nSO']
                    ci = 0
                    for i in range(NB):
                        P, SE, SO = nP[i], nSE[i], nSO[i]
                        att = atpool.tile([128, 2048], BF16, tag="att")
                        # dead half-windows of expanded singles: even singles
                        # live at t=0 (dead t=1), odd singles at t=1 (dead t=0)
                        if SE:
                            nc.gpsimd.memset(
                                att[:, 2 * P * 64:(2 * P + 2 * SE) * 64].rearrange(
                                    "p (n t k) -> p n t k", t=2, k=64)[:, :, 1, :],
                                0.0)
                        if SO:
                            o0 = 2 * (P + SE)
                            nc.gpsimd.memset(
                                att[:, o0 * 64:(o0 + 2 * SO) * 64].rearrange(
                                    "p (n t k) -> p n t k", t=2, k=64)[:, :, 0, :],
                                0.0)
                        for sc in score_chunks[i]:
                            jlist = sc['jlist']
                            nbk = len(jlist)
                            npr = sc['npr']
                            p0 = sc['p0']
                            ci += 1
                            spt = ps_mm.tile([128, 512], F32, tag="mm")
                            # QK^T for both heads in one matmul each (qbd)
                            mm_list = []
                            pos = 0
                            for (j0, rl) in _runs(jlist):
                                mm_list.append((pos, j0, rl))
                                pos += rl
                            for mi, (pos, j0, rl) in enumerate(mm_list):
                                nc.tensor.matmul(
                                    spt[:, pos * 64:(pos + rl) * 64],
                                    qbd[:, i * 128:(i + 1) * 128],
                                    kT[:, j0 * 64:(j0 + rl) * 64],
                                    start=(mi == 0),
                                    stop=(mi == len(mm_list) - 1))
                            ex = epool.tile([128, 512], BF16, tag="exp")
                            nc.scalar.activation(ex[:, 0:nbk * 64], spt[:, 0:nbk * 64],
                                                 AF.Exp)
                            den = dpool.tile([128, 8], F32, tag="den")
                            rec = dpool.tile([128, 8], F32, tag="rec")
                            exh = epool.tile([128, 256], BF16, tag="exh")
                            exv = ex[:, 0:nbk * 64].rearrange(
                                "p (n k) -> p n k", k=64)
                            nc.vector.tensor_tensor(
                                exh[:, 0:nbk * 32].rearrange(
                                    "p (n k) -> p n k", k=32),
                                exv[:, :, 0:32], exv[:, :, 32:64], ALU.add)
                            nc.vector.tensor_reduce(
                                den[:, 0:nbk],
                                exh[:, 0:nbk * 32].rearrange("p (n k) -> p n k", k=32),
                                axis=AX.X, op=ALU.add)
                            nc.vector.reciprocal(rec[:, 0:nbk], den[:, 0:nbk])
                            # normalize into the window layout: pairs part is
                            # contiguous; single s (combined rank k) lives at
                            # window block 2k + parity (t=0 even, t=1 odd)
                            nse_, nso_ = sc['nse'], sc['nso']
                            neng = nc.gpsimd
                            if npr:
                                neng.tensor_tensor(
                                    att[:, p0 * 64:(p0 + 2 * npr) * 64].rearrange(
                                        "p (n k) -> p n k", k=64),
                                    ex[:, 0:2 * npr * 64].rearrange(
                                        "p (n k) -> p n k", k=64),
                                    rec[:, 0:2 * npr, None].to_broadcast(
                                        (128, 2 * npr, 64)),
                                    ALU.mult)
                            csub = 2 * npr      # compact offset within chunk
                            w0 = 2 * p0 - 2 * P + 4 * npr  # window pos of 1st single
                            for nss, tpar in ((nse_, 0), (nso_, 1)):
                                if not nss:
                                    continue
                                neng.tensor_tensor(
                                    att[:, w0 * 64:(w0 + 2 * nss) * 64].rearrange(
                                        "p (n t k) -> p n t k", t=2,
                                        k=64)[:, :, tpar, :],
                                    ex[:, csub * 64:(csub + nss) * 64].rearrange(
                                        "p (n k) -> p n k", k=64),
                                    rec[:, csub:csub + nss, None].to_broadcast(
                                        (128, nss, 64)),
                                    ALU.mult)
                                csub += nss
                                w0 += 2 * nss
                        # -------- one xbar DMA transpose per row ------------
                        W = P + SE + SO
                        s0 = i * MAXW
                        nc.sync.dma_start_transpose(
                            aTall[:, s0 * 128:(s0 + W) * 128].rearrange(
                                "p (n q) -> p n q", q=128),
                            att[:, 0:W * 128])

                    # -------- Phase B --------
                    if stage == 'phaseA':
                        prb2 = epool.tile([128, 512], F32, tag="prbA")
                        nc.vector.tensor_copy(prb2[:], aTall[:, 0:1024].bitcast(F32))
                        nc.sync.dma_start(
                            y_out.ap()[hp * 128:(hp + 1) * 128, 0:512], prb2[:])
                        continue
                    # merged run matmuls: every entry is a full-128 matmul
                    # against the V pair tile c=j//2 (the dead half-window of
                    # an expanded single is zeroed, contributing nothing).
                    # Rows with equal slot rank and consecutive i read aTall
                    # with a regular MAXW*128 stride and write consecutive
                    # otp columns.
                    aTv = aTall[:, 0:NB * MAXW * 128].rearrange(
                        "p (i q) -> p i q", q=MAXW * 128)
                    groups = {}
                    for i in range(NB):
                        for e_idx, ent in enumerate(entries[i]):
                            kind, j = ent
                            groups.setdefault(j // 2, []).append((i, e_idx))
                    runs_all = []
                    for c in sorted(groups):
                        for (i, r) in sorted(groups[c]):
                            lr = runs_all[-1] if runs_all else None
                            if (lr is not None and lr[0] == c
                                    and i == lr[1] + lr[3] and r == lr[2]
                                    and i % 8 != 0):
                                lr[3] += 1
                            else:
                                runs_all.append([c, i, r, 1])
                    otp = ps_out.tile([128, S], F32, tag="ot")
                    for lh, ob in ((0, 0), (1, 64)):
                        h = 2 * hp + lh
                        first_mm, last_mm = {}, {}
                        for mi, (c, i0, r, n) in enumerate(runs_all):
                            bk = i0 // 8
                            if bk not in first_mm:
                                first_mm[bk] = mi
                            last_mm[bk] = mi
                        for mi, (c, i0, r, n) in enumerate(runs_all):
                            bk = i0 // 8
                            st = (first_mm[bk] == mi)
                            sp = (last_mm[bk] == mi)
                            oap = otp[ob:ob + 64, i0 * 64:(i0 + n) * 64]
                            if n == 1:
                                cw = (i0 * MAXW + r) * 128 + lh * 64
                                rhs = aTall[:, cw:cw + 64]
                            else:
                                rhs = aTv[:, i0:i0 + n,
                                          r * 128 + lh * 64:r * 128 + lh * 64 + 64]
                            nc.tensor.matmul(
                                oap, V[h][:, c * 64:(c + 1) * 64],
                                rhs,
                                start=st, stop=sp, tile_position=(0, ob))
                    for sc4 in range(4):
                        nc.scalar.copy(outSB[hp][:, sc4 * 512:(sc4 + 1) * 512],
                                       otp[:, sc4 * 512:(sc4 + 1) * 512])

            otctx.__exit__(None, None, None)
            mmctx.__exit__(None, None, None)

            # ---- Wo projection (bias added host-side) ------------------------
            if stage in ('phaseA', 'phaseB'):
                if stage == 'phaseB':
                    with tc.tile_pool(name="prbB", bufs=1) as pbp:
                        prb3 = pbp.tile([128, S], F32, name="prb3")
                        nc.vector.tensor_copy(prb3[:], outSB[0][:])
                        nc.sync.dma_start(
                            y_out.ap()[0:128, :], prb3[:, 0:E])
                nc.compile()
                return nc
            with tc.tile_pool(name="wo", bufs=1) as wop, \
                 tc.tile_pool(name="yout", bufs=3) as ypool, \
                 tc.tile_pool(name="wo_ps", bufs=4, space="PSUM") as ps_wo:
                wo_sb = [wop.tile([128, E], BF16, name=f"wo{hp}", tag=f"wo{hp}") for hp in range(2)]
                for hp in range(2):
                    nc.sync.dma_start(wo_sb[hp][:],
                                      wo_in.ap()[hp * 128:(hp + 1) * 128, :])
                for st_ in range(ST):
                    yt = ypool.tile([128, E], F32, tag="yt")
                    for nchk in range(2):
                        pt = ps_wo.tile([128, 512], F32, tag="wo")
                        for hp in range(2):
                            nc.tensor.matmul(pt[:],
                                             outSB[hp][:, st_ * 128:(st_ + 1) * 128],
                                             wo_sb[hp][:, nchk * 512:(nchk + 1) * 512],
                                             start=(hp == 0), stop=(hp == 1))
                        nc.scalar.copy(yt[:, nchk * 512:(nchk + 1) * 512], pt[:])
                    nc.sync.dma_start(y_out.ap()[st_ * 128:(st_ + 1) * 128, :], yt[:])

    nc.compile()
    return nc


# ---------------------------------------------------------------- entry point

def kernel(x, Wq, bq, Wk, bk, Wv, bv, Wo, bo, block_rows, block_cols):
    global LAST_RESULTS
    from concourse.bass_utils import run_bass_kernel_spmd
    import os

    x = np.asarray(x, dtype=np.float32)
    Wq, Wk, Wv, Wo = (np.asarray(a, dtype=np.float32) for a in (Wq, Wk, Wv, Wo))
    bq, bk, bv, bo = (np.asarray(a, dtype=np.float32) for a in (bq, bk, bv, bo))

    plan = _plan(block_rows, block_cols)
    nc = _build_program(plan)

    import ml_dtypes
    BF = ml_dtypes.bfloat16
    in_maps = []
    for c in range(NCORES):
        b, g = c // 4, c % 4
        cs = slice(g * HPC * D, (g + 1) * HPC * D)
        w_qkv = np.ascontiguousarray(
            np.concatenate([Wq[:, cs], Wk[:, cs], Wv[:, cs]], axis=1)).astype(BF)
        b_qkv = np.ascontiguousarray(
            np.concatenate([bq[cs], bk[cs], bv[cs]]))
        w_o = np.ascontiguousarray(Wo[cs, :]).astype(BF)
        in_maps.append(dict(x_local=np.ascontiguousarray(x[b]),
                            w_qkv=w_qkv, b_qkv=b_qkv, w_o=w_o))

    trace = bool(int(os.environ.get("KERNEL_TRACE", "0")))
    res = run_bass_kernel_spmd(nc, in_maps, core_ids=list(range(NCORES)),
                               trace=trace)
    LAST_RESULTS = res

    y = np.zeros((B, S, E), dtype=np.float32)
    for c in range(NCORES):
        y[c // 4] += res.results[c]["y_partial"]
    y += bo[None, None, :]
    return y



# revision 71
# speedup vs baseline: 1.0367x; 1.0367x over previous
"""Block-sparse attention Trainium2 kernel (8 NeuronCores, SPMD).

Sharding: data-parallel over (batch, head-group): core c handles batch b=c//4
and heads [4*(c%4) .. 4*(c%4)+4). Block index lists are replicated (used
host-side to build the static program). Each core returns a partial
[S, E] output (its heads' contribution through Wo); the host sums the 4
partials per batch (the unshard step of the head-sharded GEMM).

Pipeline per core (all on device):
  x -> PE-transpose -> x^T -> QKV projection (weights stationary) giving
  Q^T,K^T [d,s] (f32) and V^T -> PE-transpose -> V [s,d] (bf16).
  Per head-pair (2 heads packed on 128 partitions):
    Phase A (per row-block i): scores = Q_i^T.T @ K^T runs (PSUM) -> exp (ACT)
      -> per-block denom (DVE 3D reduce) -> recip -> normalize (GPSIMD, bf16)
      -> PE-transpose 64x64 blocks -> attnT storage (bf16).
    Phase B: out^T accumulated in PSUM via V-stationary matmuls over attnT.
  Wo projection from out^T tiles (stationary) + rank-1 bias add; partial out
  DMA'd back.
"""
import numpy as np

B, S, E, H, D, BS = 2, 2048, 1024, 16, 64, 64
NB = S // BS          # 32
NCORES = 8
HPC = 4               # heads per core
NPAIRS = 628          # length of block index lists
MAXW = 16             # attnT slots reserved per row block (fixed stride)

LAST_RESULTS = None   # BassKernelResults of the most recent run (for test.py)


# ---------------------------------------------------------------- host planning

def _plan(block_rows, block_cols):
    """Static schedule shared by every head/core.

    Row layout (compact positions, for scores/exp): pairs first (each 2
    blocks), then singles ascending.  attnT window layout: pairs at the
    same positions; single k expands to 2 blocks (real + dead) so each
    entry is one aligned 128-col xbar window -> ONE dma transpose per row.

    Returns per-row structures:
      J[i]            sorted col-block list
      entries[i]      ('pair', j_even) then ('single', j) ascending
      score_chunks[i] <=8 compact blocks each: jlist, npr (pairs in chunk),
                      e0 (first entry idx), p0 (first compact position)
      col[i][e]       attnT slot (128-col window) of entry e
      nP[i], nS[i]    pair / single counts
      ncols_total     total slots
    """
    J = [[] for _ in range(NB)]
    for r, c in zip(np.asarray(block_rows).tolist(), np.asarray(block_cols).tolist()):
        J[int(r)].append(int(c))
    for i in range(NB):
        J[i].sort()

    entries = [[] for _ in range(NB)]
    score_chunks = [[] for _ in range(NB)]
    nP = [0] * NB
    nSE = [0] * NB
    nSO = [0] * NB
    for i in range(NB):
        js = J[i]
        pairs, sE, sO = [], [], []
        t = 0
        while t < len(js):
            if t + 1 < len(js) and js[t] % 2 == 0 and js[t + 1] == js[t] + 1:
                pairs.append(('pair', js[t]))
                t += 2
            elif js[t] % 2 == 0:
                sE.append(('se', js[t]))
                t += 1
            else:
                sO.append(('so', js[t]))
                t += 1
        nP[i], nSE[i], nSO[i] = len(pairs), len(sE), len(sO)
        walk = pairs + sE + sO
        entries[i] = walk
        # score chunks over the compact layout, <=8 blocks, pair-aligned
        cur, cur_blocks = [], 0
        sc_list = []
        for ent in walk:
            nb = 2 if ent[0] == 'pair' else 1
            if cur_blocks + nb > 8:
                sc_list.append(cur)
                cur, cur_blocks = [], 0
            cur.append(ent)
            cur_blocks += nb
        if cur:
            sc_list.append(cur)
        e0 = 0
        p0 = 0
        for sc in sc_list:
            jlist = []
            npr = sum(1 for e in sc if e[0] == 'pair')
            nse = sum(1 for e in sc if e[0] == 'se')
            nso = sum(1 for e in sc if e[0] == 'so')
            for ent in sc:
                jlist.append(ent[1])
                if ent[0] == 'pair':
                    jlist.append(ent[1] + 1)
            score_chunks[i].append(dict(jlist=jlist, npr=npr, nse=nse, nso=nso,
                                        e0=e0, p0=p0))
            e0 += len(sc)
            p0 += len(jlist)
        assert len(walk) <= MAXW, (i, len(walk))

    # fixed row stride MAXW slots: slot(i, e) = i*MAXW + e
    col = [dict() for _ in range(NB)]
    for i in range(NB):
        for e_idx, ent in enumerate(entries[i]):
            col[i][e_idx] = i * MAXW + e_idx
    return dict(J=J, entries=entries, score_chunks=score_chunks, col=col,
                nP=nP, nSE=nSE, nSO=nSO, ncols_total=NB * MAXW)


def _runs(jlist):
    """Maximal consecutive runs [(j0, n), ...] in a sorted j list."""
    runs = []
    for j in jlist:
        if runs and j == runs[-1][0] + runs[-1][1]:
            runs[-1][1] += 1
        else:
            runs.append([j, 1])
    return [(a, b) for a, b in runs]


# ---------------------------------------------------------------- bass program

def _build_program(plan, stage='full'):
    import concourse.bacc as bacc
    import concourse.mybir as mybir
    from concourse.tile import TileContext
    from concourse import masks

    F32 = mybir.dt.float32
    F32R = mybir.dt.float32r
    BF16 = mybir.dt.bfloat16
    AF = mybir.ActivationFunctionType
    ALU = mybir.AluOpType
    AX = mybir.AxisListType

    nc = bacc.Bacc("TRN2", target_bir_lowering=False, debug=False)

    x_in = nc.dram_tensor("x_local", [S, E], F32, kind="ExternalInput")
    wqkv_in = nc.dram_tensor("w_qkv", [E, 3 * HPC * D], BF16, kind="ExternalInput")
    bqkv_in = nc.dram_tensor("b_qkv", [3 * HPC * D], F32, kind="ExternalInput")
    wo_in = nc.dram_tensor("w_o", [HPC * D, E], BF16, kind="ExternalInput")
    y_out = nc.dram_tensor("y_partial", [S, E], F32, kind="ExternalOutput")

    NT = 3 * HPC * D // 128      # 6 qkv n-tiles
    KT = E // 128                # 8 contraction tiles
    ST = S // 128                # 16 s tiles

    ncols_total = plan['ncols_total']
    entries, score_chunks, col = plan['entries'], plan['score_chunks'], plan['col']
    J = plan['J']

    with TileContext(nc) as tc:
        with tc.tile_pool(name="const", bufs=1) as cpool, \
             tc.tile_pool(name="qkvT", bufs=1) as qpool, \
             tc.tile_pool(name="vnorm", bufs=1) as vpool, \
             tc.tile_pool(name="outsb", bufs=1) as opool:

            idf = cpool.tile([128, 128], F32)
            masks.make_identity(nc, idf[:])
            idb = cpool.tile([128, 128], BF16)
            masks.make_identity(nc, idb[:])
            ones_t = cpool.tile([1, 128], BF16)
            nc.vector.memset(ones_t[:], 1.0)
            zrow = cpool.tile([1, 64], BF16)
            nc.vector.memset(zrow[:], 0.0)
            bqkv_sb = cpool.tile([128, NT], F32)
            nc.sync.dma_start(bqkv_sb[:], bqkv_in.ap().rearrange("(t p) -> p t", p=128))
            bsc = cpool.tile([128, NT], F32)
            # q biases (tiles 0,1) pre-scaled by 1/sqrt(D)
            nc.scalar.mul(bsc[:, 0:2], bqkv_sb[:, 0:2], 1.0 / float(np.sqrt(D)))
            nc.scalar.copy(bsc[:, 2:NT], bqkv_sb[:, 2:NT])


            # ---- x -> x^T ----------------------------------------------------
            qkvT = [qpool.tile([128, S], BF16, name=f"qkvT{t}", tag=f"qkvT{t}") for t in range(NT)]
            trctx = tc.tile_pool(name="tr_ps", bufs=2, space="PSUM")
            ps_tr = trctx.__enter__()
            with tc.tile_pool(name="xload", bufs=3) as xpool, \
                 tc.tile_pool(name="xT", bufs=1) as xtp, \
                 tc.tile_pool(name="wq", bufs=1) as wpool:
                xT = [xtp.tile([128, S], BF16, name=f"xT{k}", tag=f"xT{k}") for k in range(KT)]
                for m in range(ST):
                    xs = xpool.tile([128, E], F32, tag="xs")
                    nc.sync.dma_start(xs[:], x_in.ap()[m * 128:(m + 1) * 128, :])
                    xb = xpool.tile([128, E], BF16, tag="xb")
                    nc.vector.tensor_copy(xb[:], xs[:])
                    for k in range(KT):
                        tp = ps_tr.tile([128, 512], BF16, tag="tr")
                        nc.tensor.transpose(tp[:, 0:128], xb[:, k * 128:(k + 1) * 128],
                                            idb[:])
                        eng = nc.vector if (k % 2 == 0) else nc.scalar
                        if eng is nc.vector:
                            nc.vector.tensor_copy(xT[k][:, m * 128:(m + 1) * 128],
                                                  tp[:, 0:128])
                        else:
                            nc.scalar.copy(xT[k][:, m * 128:(m + 1) * 128],
                                           tp[:, 0:128])

                # ---- QKV projection (weights stationary) --------------------
                wsb = [wpool.tile([128, 3 * HPC * D], BF16, name=f"w{k}", tag=f"w{k}")
                       for k in range(KT)]
                for k in range(KT):
                    nc.sync.dma_start(wsb[k][:], wqkv_in.ap()[k * 128:(k + 1) * 128, :])
                with tc.tile_pool(name="pj_ps", bufs=4, space="PSUM") as ps_pj:
                    for t in range(NT):
                        scale = 1.0 / float(np.sqrt(D)) if t < 2 else 1.0
                        for sc in range(S // 512):
                            pt = ps_pj.tile([128, 512], F32, tag="pj")
                            for k in range(KT):
                                nc.tensor.matmul(pt[:],
                                                 wsb[k][:, t * 128:(t + 1) * 128],
                                                 xT[k][:, sc * 512:(sc + 1) * 512],
                                                 start=(k == 0), stop=(k == KT - 1))
                            nc.scalar.activation(qkvT[t][:, sc * 512:(sc + 1) * 512],
                                                 pt[:], AF.Identity,
                                                 bias=bsc[:, t:t + 1], scale=scale)

            # ---- V^T -> V (normal layout, bf16) ------------------------------
            # Full-width transposes (both heads of a pair at once); pair tile
            # c holds keys of block 2c at partitions 0:64 and block 2c+1 at
            # 64:128 (odd-single matmuls read the high half directly).
            V = [vpool.tile([128, NB // 2 * D], BF16, name=f"V{h}", tag=f"V{h}") for h in range(HPC)]
            for vp in range(2):                 # head pairs (0,1) and (2,3)
                vt = qkvT[4 + vp]
                for c4 in range(0, NB // 2, 4):  # 4 s-chunks per psum tile
                    tp = ps_tr.tile([128, 512], BF16, tag="tr")
                    for u in range(4):
                        c = c4 + u
                        nc.tensor.transpose(tp[:, u * 128:(u + 1) * 128],
                                            vt[:, c * 128:(c + 1) * 128], idb[:])
                    for lh in range(2):
                        src = tp[:, 0:512].rearrange("p (n q) -> p n q", q=128)[
                            :, :, lh * 64:(lh + 1) * 64]
                        dst = V[2 * vp + lh][:, c4 * 64:(c4 + 4) * 64].rearrange(
                            "p (n q) -> p n q", q=64)
                        if lh == 0:
                            nc.scalar.copy(dst, src)
                        else:
                            nc.vector.tensor_copy(dst, src)

            trctx.__exit__(None, None, None)   # free transpose PSUM banks
            mmctx = tc.tile_pool(name="mm_ps", bufs=4, space="PSUM")
            ps_mm = mmctx.__enter__()
            otctx = tc.tile_pool(name="ot_ps", bufs=1, space="PSUM")
            ps_out = otctx.__enter__()

            outSB = [opool.tile([128, S], BF16, name=f"outSB{hp}", tag=f"outSB{hp}") for hp in range(2)]

            if stage == 'proj':
                prb = opool.tile([128, E], F32, name="prb")
                nc.vector.tensor_copy(prb[:], qkvT[0][:, 0:E])
                nc.sync.dma_start(y_out.ap()[0:128, :], prb[:])
                nc.vector.tensor_copy(prb[:], V[3][:, 0:E].bitcast(F32).broadcast_to([128, E]) if False else qkvT[5][:, 0:E])
                nc.sync.dma_start(y_out.ap()[128:256, :], prb[:])

            # ---- attention per head pair ------------------------------------
            # attnT storage: one 128-col slot per entry ([2 key-blocks] x
            # [head-a q | head-b q]); slots filled by SBUF->SBUF DMA
            # transposes (xbar), no PE transposes / PSUM copies.
            with tc.tile_pool(name="attnT", bufs=1) as apool:
             if stage != 'proj':
              for hp in range(2):
                ha, hb_ = 2 * hp, 2 * hp + 1
                qT = qkvT[hp]          # [128, S] heads (ha at 0:64, hb at 64:128)
                kT = qkvT[2 + hp]
                aTall = apool.tile([128, ncols_total * 128], BF16,
                                   name=f"aTall{hp}", tag="aTall")
                # block-diagonal Q: qbd[:, i-block] = [[qa_i, 0], [0, qb_i]]
                # so one matmul computes both heads' scores (zeros kill the
                # cross-head contraction terms)
                qbd = apool.tile([128, NB * 128], BF16, name=f"qbd{hp}",
                                 tag="qbd")
                nc.vector.memset(qbd[:], 0.0)
                nc.vector.tensor_copy(
                    qbd[0:64, 0:NB * 128].rearrange(
                        "p (n q) -> p n q", q=128)[:, :, 0:64],
                    qT[0:64, 0:S].rearrange("p (n q) -> p n q", q=64))
                nc.scalar.copy(
                    qbd[64:128, 0:NB * 128].rearrange(
                        "p (n q) -> p n q", q=128)[:, :, 64:128],
                    qT[64:128, 0:S].rearrange("p (n q) -> p n q", q=64))

                with tc.tile_pool(name=f"exp{hp}", bufs=6) as epool, \
                     tc.tile_pool(name=f"att{hp}", bufs=3) as atpool, \
                     tc.tile_pool(name=f"den{hp}", bufs=8) as dpool:
                    # -------- Phase A --------
                    nP, nSE, nSO = plan['nP'], plan['nSE'], plan['nSO']
                    ci = 0
                    for i in range(NB):
                        P, SE, SO = nP[i], nSE[i], nSO[i]
                        att = atpool.tile([128, 2048], BF16, tag="att")
                        # dead half-windows of expanded singles: even singles
                        # live at t=0 (dead t=1), odd singles at t=1 (dead t=0)
                        if SE:
                            nc.gpsimd.memset(
                                att[:, 2 * P * 64:(2 * P + 2 * SE) * 64].rearrange(
                                    "p (n t k) -> p n t k", t=2, k=64)[:, :, 1, :],
                                0.0)
                        if SO:
                            o0 = 2 * (P + SE)
                            nc.gpsimd.memset(
                                att[:, o0 * 64:(o0 + 2 * SO) * 64].rearrange(
                                    "p (n t k) -> p n t k", t=2, k=64)[:, :, 0, :],
                                0.0)
                        for sc in score_chunks[i]:
                            jlist = sc['jlist']
                            nbk = len(jlist)
                            npr = sc['npr']
                            p0 = sc['p0']
                            ci += 1
                            spt = ps_mm.tile([128, 512], F32, tag="mm")
                            # QK^T for both heads in one matmul each (qbd)
                            mm_list = []
                            pos = 0
                            for (j0, rl) in _runs(jlist):
                                mm_list.append((pos, j0, rl))
                                pos += rl
                            for mi, (pos, j0, rl) in enumerate(mm_list):
                                nc.tensor.matmul(
                                    spt[:, pos * 64:(pos + rl) * 64],
                                    qbd[:, i * 128:(i + 1) * 128],
                                    kT[:, j0 * 64:(j0 + rl) * 64],
                                    start=(mi == 0),
                                    stop=(mi == len(mm_list) - 1))
                            ex = epool.tile([128, 512], F32, tag="exp")
                            nc.scalar.activation(ex[:, 0:nbk * 64], spt[:, 0:nbk * 64],
                                                 AF.Exp)
                            den = dpool.tile([128, 8], F32, tag="den")
                            rec = dpool.tile([128, 8], F32, tag="rec")
                            nc.vector.tensor_reduce(
                                den[:, 0:nbk],
                                ex[:, 0:nbk * 64].rearrange("p (n k) -> p n k", k=64),
                                axis=AX.X, op=ALU.add)
                            nc.vector.reciprocal(rec[:, 0:nbk], den[:, 0:nbk])
                            # normalize into the window layout: pairs part is
                            # contiguous; single s (combined rank k) lives at
                            # window block 2k + parity (t=0 even, t=1 odd)
                            nse_, nso_ = sc['nse'], sc['nso']
                            neng = nc.gpsimd
                            if npr:
                                neng.tensor_tensor(
                                    att[:, p0 * 64:(p0 + 2 * npr) * 64].rearrange(
                                        "p (n k) -> p n k", k=64),
                                    ex[:, 0:2 * npr * 64].rearrange(
                                        "p (n k) -> p n k", k=64),
                                    rec[:, 0:2 * npr, None].to_broadcast(
                                        (128, 2 * npr, 64)),
                                    ALU.mult)
                            csub = 2 * npr      # compact offset within chunk
                            w0 = 2 * p0 - 2 * P + 4 * npr  # window pos of 1st single
                            for nss, tpar in ((nse_, 0), (nso_, 1)):
                                if not nss:
                                    continue
                                neng.tensor_tensor(
                                    att[:, w0 * 64:(w0 + 2 * nss) * 64].rearrange(
                                        "p (n t k) -> p n t k", t=2,
                                        k=64)[:, :, tpar, :],
                                    ex[:, csub * 64:(csub + nss) * 64].rearrange(
                                        "p (n k) -> p n k", k=64),
                                    rec[:, csub:csub + nss, None].to_broadcast(
                                        (128, nss, 64)),
                                    ALU.mult)
                                csub += nss
                                w0 += 2 * nss
                        # -------- one xbar DMA transpose per row ------------
                        W = P + SE + SO
                        s0 = i * MAXW
                        nc.sync.dma_start_transpose(
                            aTall[:, s0 * 128:(s0 + W) * 128].rearrange(
                                "p (n q) -> p n q", q=128),
                            att[:, 0:W * 128])

                    # -------- Phase B --------
                    if stage == 'phaseA':
                        prb2 = epool.tile([128, 512], F32, tag="prbA")
                        nc.vector.tensor_copy(prb2[:], aTall[:, 0:1024].bitcast(F32))
                        nc.sync.dma_start(
                            y_out.ap()[hp * 128:(hp + 1) * 128, 0:512], prb2[:])
                        continue
                    # merged run matmuls: every entry is a full-128 matmul
                    # against the V pair tile c=j//2 (the dead half-window of
                    # an expanded single is zeroed, contributing nothing).
                    # Rows with equal slot rank and consecutive i read aTall
                    # with a regular MAXW*128 stride and write consecutive
                    # otp columns.
                    aTv = aTall[:, 0:NB * MAXW * 128].rearrange(
                        "p (i q) -> p i q", q=MAXW * 128)
                    groups = {}
                    for i in range(NB):
                        for e_idx, ent in enumerate(entries[i]):
                            kind, j = ent
                            groups.setdefault(j // 2, []).append((i, e_idx))
                    runs_all = []
                    for c in sorted(groups):
                        for (i, r) in sorted(groups[c]):
                            lr = runs_all[-1] if runs_all else None
                            if (lr is not None and lr[0] == c
                                    and i == lr[1] + lr[3] and r == lr[2]
                                    and i % 8 != 0):
                                lr[3] += 1
                            else:
                                runs_all.append([c, i, r, 1])
                    otp = ps_out.tile([128, S], F32, tag="ot")
                    for lh, ob in ((0, 0), (1, 64)):
                        h = 2 * hp + lh
                        first_mm, last_mm = {}, {}
                        for mi, (c, i0, r, n) in enumerate(runs_all):
                            bk = i0 // 8
                            if bk not in first_mm:
                                first_mm[bk] = mi
                            last_mm[bk] = mi
                        for mi, (c, i0, r, n) in enumerate(runs_all):
                            bk = i0 // 8
                            st = (first_mm[bk] == mi)
                            sp = (last_mm[bk] == mi)
                            oap = otp[ob:ob + 64, i0 * 64:(i0 + n) * 64]
                            if n == 1:
                                cw = (i0 * MAXW + r) * 128 + lh * 64
                                rhs = aTall[:, cw:cw + 64]
                            else:
                                rhs = aTv[:, i0:i0 + n,
                                          r * 128 + lh * 64:r * 128 + lh * 64 + 64]
                            nc.tensor.matmul(
                                oap, V[h][:, c * 64:(c + 1) * 64],
                                rhs,
                                start=st, stop=sp, tile_position=(0, ob))
                    for sc4 in range(4):
                        nc.scalar.copy(outSB[hp][:, sc4 * 512:(sc4 + 1) * 512],
                                       otp[:, sc4 * 512:(sc4 + 1) * 512])

            otctx.__exit__(None, None, None)
            mmctx.__exit__(None, None, None)

            # ---- Wo projection (bias added host-side) ------------------------
            if stage in ('phaseA', 'phaseB'):
                if stage == 'phaseB':
                    with tc.tile_pool(name="prbB", bufs=1) as pbp:
                        prb3 = pbp.tile([128, S], F32, name="prb3")
                        nc.vector.tensor_copy(prb3[:], outSB[0][:])
                        nc.sync.dma_start(
                            y_out.ap()[0:128, :], prb3[:, 0:E])
                nc.compile()
                return nc
            with tc.tile_pool(name="wo", bufs=1) as wop, \
                 tc.tile_pool(name="yout", bufs=3) as ypool, \
                 tc.tile_pool(name="wo_ps", bufs=4, space="PSUM") as ps_wo:
                wo_sb = [wop.tile([128, E], BF16, name=f"wo{hp}", tag=f"wo{hp}") for hp in range(2)]
                for hp in range(2):
                    nc.sync.dma_start(wo_sb[hp][:],
                                      wo_in.ap()[hp * 128:(hp + 1) * 128, :])
                for st_ in range(ST):
                    yt = ypool.tile([128, E], F32, tag="yt")
                    for nchk in range(2):
                        pt = ps_wo.tile([128, 512], F32, tag="wo")
                        for hp in range(2):
                            nc.tensor.matmul(pt[:],
                                             outSB[hp][:, st_ * 128:(st_ + 1) * 128],
                                             wo_sb[hp][:, nchk * 512:(nchk + 1) * 512],
                                             start=(hp == 0), stop=(hp == 1))
                        nc.scalar.copy(yt[:, nchk * 512:(nchk + 1) * 512], pt[:])
                    nc.sync.dma_start(y_out.ap()[st_ * 128:(st_ + 1) * 128, :], yt[:])

    nc.compile()
    return nc


# ---------------------------------------------------------------- entry point

def kernel(x, Wq, bq, Wk, bk, Wv, bv, Wo, bo, block_rows, block_cols):
    global LAST_RESULTS
    from concourse.bass_utils import run_bass_kernel_spmd
    import os

    x = np.asarray(x, dtype=np.float32)
    Wq, Wk, Wv, Wo = (np.asarray(a, dtype=np.float32) for a in (Wq, Wk, Wv, Wo))
    bq, bk, bv, bo = (np.asarray(a, dtype=np.float32) for a in (bq, bk, bv, bo))

    plan = _plan(block_rows, block_cols)
    nc = _build_program(plan)

    import ml_dtypes
    BF = ml_dtypes.bfloat16
    in_maps = []
    for c in range(NCORES):
        b, g = c // 4, c % 4
        cs = slice(g * HPC * D, (g + 1) * HPC * D)
        w_qkv = np.ascontiguousarray(
            np.concatenate([Wq[:, cs], Wk[:, cs], Wv[:, cs]], axis=1)).astype(BF)
        b_qkv = np.ascontiguousarray(
            np.concatenate([bq[cs], bk[cs], bv[cs]]))
        w_o = np.ascontiguousarray(Wo[cs, :]).astype(BF)
        in_maps.append(dict(x_local=np.ascontiguousarray(x[b]),
                            w_qkv=w_qkv, b_qkv=b_qkv, w_o=w_o))

    trace = bool(int(os.environ.get("KERNEL_TRACE", "0")))
    res = run_bass_kernel_spmd(nc, in_maps, core_ids=list(range(NCORES)),
                               trace=trace)
    LAST_RESULTS = res

    y = np.zeros((B, S, E), dtype=np.float32)
    for c in range(NCORES):
        y[c // 4] += res.results[c]["y_partial"]
    y += bo[None, None, :]
    return y

